# revision 1
# baseline (speedup 1.0000x reference)
"""Trainium2 Bass kernel for nn_DeconvLayer (cascaded order-16 IIR along rows).

Math: reference applies a causal order-16 linear recurrence with taps h
(then again with reversed taps) along each row of a [4096, 4096] f32 matrix,
with the first K=16 outputs forced to zero and x[i] entering only for i >= K.

This is equivalent to  y = g (*) x_masked  where x_masked zeroes columns
0..15 and g is the (rapidly decaying) impulse response of the cascaded
filter. |g[t]| < 4e-7 for t >= 129 for stable taps (|h| ~ 0.05), so a
truncated FIR of ~256 taps is exact to f32 precision.

On-device layout: the per-row convolution contracts along time, so time must
sit on SBUF partitions. Rows are sharded 512/core across 8 cores. Per core:
  - plain fp16 loads (rows on partitions), TensorE 128x128 transposes build
    U_b[t, r] = x[r, 128*b + t] in PSUM, DVE copies them to SBUF
  - TensorE computes out[r, 128*ot + m] = sum_d sum_k U_{ot-d}[k, r] * G_d[k, m]
    with G_d[k, m] = g[128*d + m - k]  (Toeplitz slabs, D=2), f32 PSUM accum
  - DVE/ACT cast-copy PSUM->SBUF fp16, DMA out fp16 (host upcasts to f32)
"""

import os
import time

import numpy as np

# the trace path needs antenv.axon_hooks, absent in this container; make
# sure a stray BASS_TRACE in the caller's env can't break execution
os.environ.setdefault("BASS_NEVER_TRACE", "1")

import concourse.bass as bass
import concourse.mybir as mybir
from concourse.bass_utils import run_bass_kernel_spmd
from concourse.tile import TileContext

N_CORES = 8
ROWS = 4096
COLS = 4096
ROWS_PER_CORE = ROWS // N_CORES  # 512
K_TAPS = 16
D = 2          # matmul depth: output tile ot reads input tiles ot, ot-1
T_FIR = 256    # taps kept when building G (effective coverage per output m: m+129)
NT = COLS // 128   # 32 time tiles
NQ = NT // 4       # 8 column panels of 512
NRC = ROWS_PER_CORE // 128  # 4 row chunks per core

_F16 = mybir.dt.float16
_F32 = mybir.dt.float32

# schedule-tuning knobs (swept in sim; defaults are the tuned values)
P_PT = 3        # transpose-PSUM pool banks
P_PO = 5        # conv-PSUM pool banks
P_U = 2         # transposed-quad SBUF slots per tag
P_Y = 3         # output-panel SBUF slots
RC0_PIECES = 8  # x-load pieces for the first row chunk
RC_PIECES = 2   # x-load pieces for later row chunks
COPY_MOD = 5    # conv-copy on DVE when q % COPY_MOD == 0, else ACT
# Boundary split: output columns [S, 128) of each 128-tile are covered by a
# SINGLE matmul from one aligned input tile (their taps [0, S] fit inside
# it); only columns [0, S) need the two-tile pair. Conv cost per tile drops
# from 256 to 128+S cycles. Tap-truncation to S+1 taps at the boundary
# columns adds 3.1e-5 L2 (negligible vs the 2.1e-4 fp16 input rounding).
P_S = 80


def _impulse_response(h: np.ndarray, n: int) -> np.ndarray:
    """Impulse response of v[i] = x[i] + sum_j h[j] v[i-1-j], float64."""
    g = np.zeros(n, np.float64)
    g[0] = 1.0
    K = len(h)
    for t in range(1, n):
        lo = max(0, t - K)
        g[t] = np.dot(h[: t - lo], g[t - 1 : lo - 1 if lo > 0 else None : -1])
    return g


def _build_g_cat(h32: np.ndarray) -> np.ndarray:
    """[128, 256] fp16 Toeplitz slabs: [G_b1 | G_b0 | G_s] (widths S, S, 128-S).

    G_loc[k, m]  = g[m - k]        (own-tile taps [0, m], all 128 cols)
    G_deep[k, m] = g[128 + m - k]  (prev-tile taps [m+1, S]; only the first
                                    S cols have deep taps <= S, rest dropped)
    """
    S = P_S
    h = h32.astype(np.float64)
    g1 = _impulse_response(h, T_FIR)
    g2 = _impulse_response(h[::-1], T_FIR)
    gc = np.convolve(g1, g2)[:T_FIR]
    kk = np.arange(128)[:, None]

    def toe(offs, width, tmax):
        mm = np.arange(width)[None, :]
        t = offs + mm - kk
        valid = (t >= 0) & (t <= tmax) & (t < T_FIR)
        return np.where(valid, gc[np.clip(t, 0, T_FIR - 1)], 0.0)

    g_loc = toe(0, 128, T_FIR - 1)  # taps [0 .. m]
    g_deep = toe(128, S, P_S)       # taps [m+1 .. S]
    out = np.zeros((128, 128 * D))
    out[:, : 128 + S] = np.concatenate([g_loc, g_deep], axis=1)
    return out.astype(np.float16)


def _build_program(legalize: bool = True) -> bass.Bass:
    """Per-core program.

    Engine roles (balanced so each engine's busy time is ~16-21us and the
    schedule is PE-bound; _legalize_waits post-pass keeps every instruction
    within this walrus' one-semaphore-wait capacity):
      - GPSIMD (SWDGE): x loads (pieces, for early pipeline fill)
      - PE: 128x128 fp16 transposes + Toeplitz conv matmuls (f32 PSUM)
      - DVE: transposed-tile PSUM->SBUF copies (uint32-bitcast) + 1/4 of
        the conv-result cast-copies
      - ACT: 3/4 of the conv-result f32->fp16 cast-copies
      - SP (HWDGE): const loads + stores (last row-chunk's stores split
        with Pool to parallelize the tail)
    """
    nc = bass.Bass()
    x = nc.dram_tensor("x", [ROWS_PER_CORE, COLS], _F16, kind="ExternalInput")
    g = nc.dram_tensor("g", [128, 128 * D], _F16, kind="ExternalInput")
    ident = nc.dram_tensor("ident", [128, 128], _F16, kind="ExternalInput")
    # fp16 output: halves store bytes (the HBM floor dominates e2e); costs
    # ~1e-4 extra rounding (host upcasts to f32)
    y = nc.dram_tensor("y", [ROWS_PER_CORE, COLS], _F16, kind="ExternalOutput")

    with TileContext(nc) as tc:
        with (
            tc.tile_pool(name="cpool", bufs=1) as cpool,
            tc.tile_pool(name="xpool", bufs=1) as xpool,
            tc.tile_pool(name="upool", bufs=P_U) as upool,
            tc.tile_pool(name="ptpool", bufs=P_PT, space="PSUM") as ptpool,
            tc.tile_pool(name="popool", bufs=P_PO, space="PSUM") as popool,
            tc.tile_pool(name="ypool", bufs=P_Y) as ypool,
        ):
            # ident first on SP — it gates the first transposes; g is loaded
            # after rc0's x pieces (only needed once conv starts)
            idt = cpool.tile([128, 128], _F16, tag="id")
            nc.sync.dma_start(idt[:], ident[:])
            gt = cpool.tile([128, 128 * D], _F16, tag="g")

            for rc in range(4):
                rs = slice(128 * rc, 128 * (rc + 1))
                # x loaded in pieces so early transposes start sooner; the
                # first row-chunk uses eighth-loads for fast pipeline fill
                npieces = RC0_PIECES if rc == 0 else RC_PIECES
                xph = []
                pw = COLS // npieces
                for h in range(npieces):
                    xp = xpool.tile([128, pw], _F16, tag=f"x{rc}_{h}")
                    nc.gpsimd.dma_start(xp[:], x[rs, pw * h : pw * (h + 1)])
                    xph.append(xp)
                if rc == 0:
                    nc.sync.dma_start(gt[:], g[:])

                # transpose quads: U[b][t, r] = x[rs, :][r, 128*b + t]
                u_quads = {}
                for tq in range(NQ):
                    ptt = ptpool.tile([128, 512], _F16, tag="pt")
                    for j in range(4):
                        b = 4 * tq + j
                        xp = xph[(128 * b) // pw]
                        bb = b - (128 * b) // pw * (pw // 128)
                        nc.tensor.transpose(
                            ptt[:, 128 * j : 128 * (j + 1)],
                            xp[:, 128 * bb : 128 * (bb + 1)],
                            idt[:],
                        )
                    uq = upool.tile([128, 512], _F16, tag=f"u{tq}")
                    # bitcast fp16 pairs to uint32: halves the column count
                    # (copies are column-rate-bound; uint64 fails the walrus
                    # ISA check). DVE only — the ACT activation path mangles
                    # raw integer bit patterns.
                    nc.vector.tensor_copy(
                        uq[:].bitcast(mybir.dt.uint32),
                        ptt[:].bitcast(mybir.dt.uint32),
                    )
                    u_quads[tq] = uq

                def u_slice(b, u_quads=u_quads):
                    return u_quads[b // 4][:, 128 * (b % 4) : 128 * (b % 4 + 1)]

                for pg in range(2):  # output panels of 2048 cols (4 banks)
                    gp = rc * 2 + pg
                    yp = ypool.tile([128, 2048], _F16, tag="y")
                    for qq in range(4):
                        q = 4 * pg + qq
                        pt = popool.tile([128, 512], _F32, tag="po")
                        # Per output tile j: one full-width own-tile matmul
                        # (G_loc, taps [0,m]) plus one S-wide deep-history
                        # matmul from the previous tile (G_deep, taps
                        # [m+1,S]). Own-tile writes go first (uniformly fresh
                        # after the opener's start=True bank zeroing), deep
                        # writes then accumulate uniformly.
                        S = P_S
                        plan = []
                        for j in range(4):
                            plan.append((128 * j, 128, 4 * q + j, 0))
                        for j in range(4):
                            if 4 * q + j - 1 >= 0:
                                plan.append((128 * j, S, 4 * q + j - 1, 128))
                        for i, (col, w, b, goff) in enumerate(plan):
                            nc.tensor.matmul(
                                pt[:, col : col + w],
                                lhsT=u_slice(b),
                                rhs=gt[:, goff : goff + w],
                                start=(i == 0),
                                stop=(i == len(plan) - 1),
                            )
                        # cast-copy f32 PSUM -> fp16 SBUF; mostly ACT (DVE
                        # already carries the uint32 u-copies). In the final
                        # row chunk alternate DVE/ACT so the tail copies
                        # drain in parallel instead of serializing on ACT.
                        if rc == 3:
                            dve_copy = q in (4, 6, 7)
                        else:
                            dve_copy = q % COPY_MOD == 0
                        ceng = nc.vector.tensor_copy if dve_copy else nc.scalar.copy
                        ceng(yp[:, 512 * qq : 512 * (qq + 1)], pt[:])
                    # stores on SP (loads own Pool); split the last chunk's
                    # panels across SP+Pool so the tail stores parallelize
                    c0 = 2048 * pg
                    if gp == 7:
                        nc.sync.dma_start(y[rs, c0 : c0 + 1024], yp[:, 0:1024])
                        nc.gpsimd.dma_start(
                            y[rs, c0 + 1024 : c0 + 1536], yp[:, 1024:1536]
                        )
                        nc.sync.dma_start(
                            y[rs, c0 + 1536 : c0 + 1792], yp[:, 1536:1792]
                        )
                        nc.gpsimd.dma_start(
                            y[rs, c0 + 1792 : c0 + 2048], yp[:, 1792:2048]
                        )
                    elif gp == 6:
                        nc.sync.dma_start(y[rs, c0 : c0 + 1024], yp[:, 0:1024])
                        nc.gpsimd.dma_start(
                            y[rs, c0 + 1024 : c0 + 2048], yp[:, 1024:2048]
                        )
                    else:
                        nc.sync.dma_start(y[rs, c0 : c0 + 2048], yp[:])
    if legalize:
        _legalize_waits(nc)
    return nc


def _legalize_waits(nc: bass.Bass) -> None:
    """This toolchain's walrus accepts at most ONE semaphore wait per
    instruction (Drain/EventSemaphore excepted), but Tile's semaphore
    assignment freely emits 2-3. Hoist extra waits onto injected same-engine
    NoOps placed immediately before the instruction — engines execute their
    stream serially (and a DMA trigger precedes its descriptor execution),
    so waiting earlier on the same engine preserves semantics.
    """
    for fn in nc.m.functions:
        for blk in fn.blocks:
            out = []
            changed = False
            for i in blk.instructions:
                tn = type(i).__name__
                si = i.sync_info
                cap = 2 if tn == "InstEventSemaphore" else 1
                if si is not None and len(si.on_wait) > cap:
                    waits = list(si.on_wait)
                    for w in waits[:-cap]:
                        out.append(
                            mybir.InstNoOp(
                                name=nc.get_next_instruction_name(),
                                ins=[],
                                outs=[],
                                engine=i.engine,
                                sync_info=mybir.SyncInfo(
                                    on_wait=[w], on_update=[]
                                ),
                            )
                        )
                    i.sync_info = mybir.SyncInfo(
                        on_wait=waits[-cap:], on_update=list(si.on_update)
                    )
                    changed = True
                out.append(i)
            if changed:
                blk.instructions = out


_PROGRAM = None


def kernel(**inputs: np.ndarray) -> np.ndarray:
    global _PROGRAM
    x = np.asarray(inputs["inputs"], dtype=np.float32)
    h = np.asarray(inputs["kernel"], dtype=np.float32)[0]
    assert x.shape == (ROWS, COLS) and h.shape == (K_TAPS,)

    g_cat = _build_g_cat(h)
    xm = x.astype(np.float16)
    xm[:, :K_TAPS] = 0

    if _PROGRAM is None:
        _PROGRAM = _build_program()

    ident = np.eye(128, dtype=np.float16)
    in_maps = [
        {
            "x": xm[ROWS_PER_CORE * c : ROWS_PER_CORE * (c + 1)],
            "g": g_cat,
            "ident": ident,
        }
        for c in range(N_CORES)
    ]
    # the axon-proxied device occasionally reports a transient
    # NRT_EXEC_UNIT_UNRECOVERABLE; a retry succeeds
    last_err = None
    for _ in range(3):
        try:
            res = run_bass_kernel_spmd(
                _PROGRAM, in_maps, list(range(N_CORES))
            ).results
            break
        except Exception as e:  # noqa: BLE001
            last_err = e
            time.sleep(2.0)
    else:
        raise last_err
    out = np.concatenate([res[c]["y"] for c in range(N_CORES)], axis=0)
    return out.astype(np.float32)

